# revision 21
# baseline (speedup 1.0000x reference)
"""Trainium2 Bass kernel for a 3-layer dense GCN (nn_GCN_13846974562486).

Math (reference):
    h1 = relu(adj @ (x  @ W1) + b1)   # [N, 32]
    h2 = relu(adj @ (h1 @ W2) + b2)   # [N, 48]
    h3 = relu(adj @ (h2 @ W3) + b3)   # [N, 64]
    y  = softmax(relu(mean(h3, 0) @ fcW1 + fcb1) @ fcW2 + fcb2)

Distribution: 1D row-shard of adj / output nodes over 8 cores. Each core
holds AT_c = adj[rows_c, :].T  (i.e. adj^T column-block, 128 MiB) and
computes its 2048 output rows per layer:
    O[m, f] = sum_k AT_c[k, m] * z[k, f]
as PE matmuls with the AT tile in the stationary (weight) slot — for fp32
the weight-load path moves 128 elem/cycle vs the streaming path's ~32, so
the PE stays under the DMA roofline (adj is read once per layer: the
memory-bound term, 3 x 128 MiB per core).

Between layers the [N, F] activations z_{l+1} = relu-ed h_l @ W_{l+1} are
AllGathered (2-4 MiB). The mean-pool partial sum [64] is computed on
device per core; the 8 partials and the tiny MLP head are combined on the
host (exact fp32, ~100 kFLOP).
"""

import os
import sys

for _p in ("/opt/trn_rl_repo", "/root/.axon_site/_ro/trn_rl_repo"):
    if os.path.isdir(_p) and _p not in sys.path:
        sys.path.insert(0, _p)

from contextlib import ExitStack

import numpy as np

import concourse.bass as bass
import concourse.mybir as mybir
import concourse.tile as tile
from concourse import bacc
from concourse.bass_utils import run_bass_kernel_spmd
from concourse.masks import make_identity

F32 = mybir.dt.float32

N = 16384           # nodes
NFEAT = 128         # input features
F1, F2, F3 = 32, 48, 64
NCORES = 8
R = N // NCORES     # rows (output nodes) per core = 2048
KT = N // 128       # k-tiles per layer = 128
MT = R // 128       # m-tiles per core = 16
KB = 32             # k-tiles per DMA chunk (2 MiB per dma_start)
KC = KT // KB       # DMA chunks per m-tile = 4

# The at input is host-preswizzled so every DMA chunk is a fully
# sequential DRAM read (16 KiB contiguous per partition):
#   at_sw[mt, kc, p, kk, m] = adj[c*R + mt*128 + m, (kc*KB + kk)*128 + p]
# declared on device as [MT*KC*128, KB*128].
AT_SHAPE = (MT * KC * 128, KB * 128)


def _ts(i, s):
    return slice(i * s, (i + 1) * s)


def _build_nc(reps=1):
    nc = bacc.Bacc(
        "TRN2", target_bir_lowering=False, debug=False, num_devices=NCORES
    )

    at = nc.dram_tensor("at", list(AT_SHAPE), F32, kind="ExternalInput")
    xt = nc.dram_tensor("xt", [NFEAT, N], F32, kind="ExternalInput")
    w1 = nc.dram_tensor("w1", [NFEAT, F1], F32, kind="ExternalInput")
    w2 = nc.dram_tensor("w2", [F1, F2], F32, kind="ExternalInput")
    w3 = nc.dram_tensor("w3", [F2, F3], F32, kind="ExternalInput")
    b1b = nc.dram_tensor("b1b", [128, F1], F32, kind="ExternalInput")
    b2b = nc.dram_tensor("b2b", [128, F2], F32, kind="ExternalInput")
    b3b = nc.dram_tensor("b3b", [128, F3], F32, kind="ExternalInput")
    out = nc.dram_tensor("out", [1, F3], F32, kind="ExternalOutput")

    # [MT, KC, 128, KB*128]; chunk (mt, kc) is 2 MiB of sequential DRAM
    at_r = at[:].rearrange("(mt kc p) q -> mt kc p q", kc=KC, p=128)

    with tile.TileContext(nc) as tc, ExitStack() as es:
        const = es.enter_context(tc.tile_pool(name="const", bufs=1))
        atp = es.enter_context(tc.tile_pool(name="atp", bufs=4))
        hp = es.enter_context(tc.tile_pool(name="hp", bufs=2))
        htp = es.enter_context(tc.tile_pool(name="htp", bufs=2))
        zlp = es.enter_context(tc.tile_pool(name="zlp", bufs=1))
        accp = es.enter_context(tc.tile_pool(name="accp", bufs=1))
        dram = es.enter_context(tc.tile_pool(name="dram", bufs=1, space="DRAM"))
        p_acc = es.enter_context(tc.tile_pool(name="p_acc", bufs=2, space="PSUM"))
        p_z = es.enter_context(tc.tile_pool(name="p_z", bufs=2, space="PSUM"))
        p_t = es.enter_context(tc.tile_pool(name="p_t", bufs=2, space="PSUM"))
        p_s = es.enter_context(tc.tile_pool(name="p_s", bufs=2, space="PSUM"))

        w1_sb = const.tile([NFEAT, F1], F32)
        w2_sb = const.tile([F1, F2], F32)
        w3_sb = const.tile([F2, F3], F32)
        b1_sb = const.tile([128, F1], F32)
        b2_sb = const.tile([128, F2], F32)
        b3_sb = const.tile([128, F3], F32)
        ones_sb = const.tile([128, 1], F32)
        ident_sb = const.tile([128, 128], F32)
        acc_sb = accp.tile([1, F3], F32)

        nc.sync.dma_start(w1_sb[:], w1[:])
        nc.sync.dma_start(w2_sb[:], w2[:])
        nc.sync.dma_start(w3_sb[:], w3[:])
        nc.sync.dma_start(b1_sb[:], b1b[:])
        nc.sync.dma_start(b2_sb[:], b2b[:])
        nc.sync.dma_start(b3_sb[:], b3b[:])
        nc.any.memset(ones_sb[:], 1.0)
        make_identity(nc, ident_sb[:])



        def z1_compute(z1_sb):
            XQ = 4096
            with tc.tile_pool(name="xtp", bufs=2) as xtp:
                for q in range(N // XQ):
                    xq_sb = xtp.tile([NFEAT, XQ], F32, tag="xq")
                    nc.sync.dma_start(xq_sb[:], xt[:, _ts(q, XQ)])
                    for jj in range(XQ // 128):
                        j = q * (XQ // 128) + jj
                        pz = p_z.tile([128, F1], F32, tag="pz")
                        nc.tensor.matmul(
                            pz[:], xq_sb[:, _ts(jj, 128)], w1_sb[:],
                            start=True, stop=True,
                        )
                        nc.vector.tensor_copy(z1_sb[:, j, :], pz[:])

        def layer(z_sb, f_in, w_sb, b_sb, f_out, znext_local, last):
            for mt in range(MT):
                pacc = p_acc.tile([128, f_in], F32, tag="pacc")
                for kc in range(KC):
                    a_sb = atp.tile([128, KB * 128], F32, tag="a")
                    nc.sync.dma_start(a_sb[:], at_r[mt, kc])
                    for kk in range(KB):
                        kt = kc * KB + kk
                        nc.tensor.matmul(
                            pacc[:],
                            a_sb[:, _ts(kk, 128)],
                            z_sb[:, kt, :],
                            start=(kt == 0),
                            stop=(kt == KT - 1),
                        )
                h_sb = hp.tile([128, f_in], F32, tag="h")
                nc.vector.tensor_tensor(
                    h_sb[:], pacc[:], b_sb[:, :f_in], mybir.AluOpType.add
                )
                nc.vector.tensor_scalar_max(h_sb[:], h_sb[:], 0.0)
                if last:
                    ps = p_s.tile([1, f_in], F32, tag="ps")
                    nc.tensor.matmul(
                        ps[:], ones_sb[:], h_sb[:], start=True, stop=True
                    )
                    nc.vector.tensor_tensor(
                        acc_sb[:], acc_sb[:], ps[:], mybir.AluOpType.add
                    )
                else:
                    pt = p_t.tile([f_in, 128], F32, tag="pt")
                    nc.tensor.transpose(pt[:], h_sb[:], ident_sb[:])
                    ht_sb = htp.tile([f_in, 128], F32, tag="ht")
                    nc.vector.tensor_copy(ht_sb[:], pt[:])
                    pz = p_z.tile([128, f_out], F32, tag="pz")
                    nc.tensor.matmul(
                        pz[:], ht_sb[:], w_sb[:], start=True, stop=True
                    )
                    nc.vector.tensor_copy(znext_local[:, mt, :], pz[:])

        def gather(znl_sb, z_in, z_out, znext_sb, g):
            # z_in is [128, MT, g]: a straight contiguous copy of the local
            # z tiles. AllGather concatenates rank blocks, so z_out is
            # [NCORES, 128, MT, g]; global k-tile (r*MT + mt) holds nodes
            # {r*R + mt*128 + p} — exactly the standard k-tiling.
            nc.sync.dma_start(z_in[:], znl_sb[:])
            nc.gpsimd.collective_compute(
                "AllGather",
                mybir.AluOpType.bypass,
                replica_groups=[list(range(NCORES))],
                ins=[z_in.opt()],
                outs=[z_out.opt()],
            )
            nc.sync.dma_start(
                znext_sb[:].rearrange("p (r mt) g -> p r mt g", r=NCORES),
                z_out[:].rearrange("r p mt g -> p r mt g"),
            )

        for _rep in range(reps):
            # collective bounce buffers (Shared tiles are single-write)
            z2_in = dram.tile([128, MT, F2], F32, tag=f"z2i{_rep}")
            z2_out = dram.tile([NCORES, 128, MT, F2], F32,
                               addr_space="Shared", tag=f"z2o{_rep}")
            z3_in = dram.tile([128, MT, F3], F32, tag=f"z3i{_rep}")
            z3_out = dram.tile([NCORES, 128, MT, F3], F32,
                               addr_space="Shared", tag=f"z3o{_rep}")
            nc.vector.memset(acc_sb[:], 0.0)
            # Pools released mid-trace must pop in LIFO order: open z3p
            # first (longest-lived), then z2p, then z1p.
            z3_es = ExitStack()
            z3p = z3_es.enter_context(tc.tile_pool(name="z3p", bufs=1))
            z2_es = ExitStack()
            z2p = z2_es.enter_context(tc.tile_pool(name="z2p", bufs=1))
            z1_es = ExitStack()
            z1p = z1_es.enter_context(tc.tile_pool(name="z1p", bufs=1))

            z1_sb = z1p.tile([128, KT, F1], F32)
            z1_compute(z1_sb)

            # ---- layer 1 ----
            z2l_sb = zlp.tile([128, MT, F2], F32, tag="z2l")
            z2_sb = z2p.tile([128, KT, F2], F32)
            layer(z1_sb, F1, w2_sb, b1_sb, F2, z2l_sb, last=False)
            z1_es.close()
            gather(z2l_sb, z2_in, z2_out, z2_sb, F2)

            # ---- layer 2 ----
            z3l_sb = zlp.tile([128, MT, F3], F32, tag="z3l")
            z3_sb = z3p.tile([128, KT, F3], F32)
            layer(z2_sb, F2, w3_sb, b2_sb, F3, z3l_sb, last=False)
            z2_es.close()
            gather(z3l_sb, z3_in, z3_out, z3_sb, F3)

            # ---- layer 3 + mean-pool partial ----
            layer(z3_sb, F3, None, b3_sb, None, None, last=True)
            z3_es.close()

        out_sb = accp.tile([1, F3], F32)
        nc.vector.tensor_copy(out_sb[:], acc_sb[:])
        nc.sync.dma_start(out[:], out_sb[:])

    nc.compile()
    return nc


_NC_CACHE = {}


def _get_nc(reps=1):
    if reps not in _NC_CACHE:
        _NC_CACHE[reps] = _build_nc(reps)
    return _NC_CACHE[reps]


def make_in_maps(x, adj, W1, W2, W3, b1, b2, b3):
    x = np.ascontiguousarray(x, dtype=np.float32)
    xt = np.ascontiguousarray(x.T)
    b1b = np.ascontiguousarray(np.broadcast_to(b1, (128, F1)), dtype=np.float32)
    b2b = np.ascontiguousarray(np.broadcast_to(b2, (128, F2)), dtype=np.float32)
    b3b = np.ascontiguousarray(np.broadcast_to(b3, (128, F3)), dtype=np.float32)
    common = {
        "xt": xt,
        "w1": np.ascontiguousarray(W1, dtype=np.float32),
        "w2": np.ascontiguousarray(W2, dtype=np.float32),
        "w3": np.ascontiguousarray(W3, dtype=np.float32),
        "b1b": b1b,
        "b2b": b2b,
        "b3b": b3b,
    }
    adj = np.asarray(adj, dtype=np.float32)
    in_maps = []
    for c in range(NCORES):
        blk = adj[c * R : (c + 1) * R, :]           # [R, N]
        # [mt, m, kc, kk, p] -> [mt, kc, p, kk, m]
        sw = blk.reshape(MT, 128, KC, KB, 128).transpose(0, 2, 4, 3, 1)
        at_c = np.ascontiguousarray(sw).reshape(AT_SHAPE)
        in_maps.append({"at": at_c, **common})
    return in_maps


def head(h3_sum, fcW1, fcb1, fcW2, fcb2):
    y = (h3_sum / np.float32(N)).astype(np.float32)
    y = np.maximum(y @ fcW1 + fcb1, np.float32(0.0))
    logits = y @ fcW2 + fcb2
    e = np.exp(logits - logits.max())
    return (e / e.sum()).astype(np.float32)


def kernel(
    x,
    adj,
    idx_map,  # unused by the reference model
    W1,
    b1,
    W2,
    b2,
    W3,
    b3,
    fcW1,
    fcb1,
    fcW2,
    fcb2,
):
    nc = _get_nc()
    in_maps = make_in_maps(x, adj, W1, W2, W3, b1, b2, b3)
    res = run_bass_kernel_spmd(nc, in_maps, core_ids=list(range(NCORES)))
    h3_sum = np.sum(
        [res.results[c]["out"][0] for c in range(NCORES)], axis=0
    ).astype(np.float32)
    return head(h3_sum, fcW1, fcb1, fcW2, fcb2)


# revision 22
# speedup vs baseline: 1.7436x; 1.7436x over previous
"""Trainium2 Bass kernel for a 3-layer dense GCN (nn_GCN_13846974562486).

Math (reference):
    h1 = relu(adj @ (x  @ W1) + b1)   # [N, 32]
    h2 = relu(adj @ (h1 @ W2) + b2)   # [N, 48]
    h3 = relu(adj @ (h2 @ W3) + b3)   # [N, 64]
    y  = softmax(relu(mean(h3, 0) @ fcW1 + fcb1) @ fcW2 + fcb2)

Distribution: 1D row-shard of adj / output nodes over 8 cores. Each core
holds a host-preswizzled copy of adj[rows_c, :]^T (128 MiB) and computes
its 2048 output rows per layer transposed:
    hT[f, m] = relu( sum_k z[k, f] * adj[row_m, k] + b[f] )
as PE matmuls with the small z-tile [128, F] in the stationary slot and
the adjacency streamed 512-wide (measured ~2.3x faster than holding the
adjacency tile stationary, whose fused fp32 weight-load doesn't
pipeline). The host swizzle makes every 2 MiB DMA chunk a fully
sequential DRAM read (measured ~290-460 GB/s/core vs ~190-260 for the
naive strided pattern). adj is read once per layer — the memory-bound
term (3 x 128 MiB per core).

The transposed layout makes bias+relu a single fused ScalarEngine
activation from PSUM, z_next = h @ W_next a direct matmul (hT is already
the needed lhsT), and the mean-pool a free-dim reduce.

Between layers the [N, F] activations are AllGathered (1.5-2 MiB per
rank-block, contiguous layout). The 8 per-core partial sums [64] and the
tiny MLP head run on the host (exact fp32, ~100 kFLOP).
"""

import os
import sys

for _p in ("/opt/trn_rl_repo", "/root/.axon_site/_ro/trn_rl_repo"):
    if os.path.isdir(_p) and _p not in sys.path:
        sys.path.insert(0, _p)

from contextlib import ExitStack

import numpy as np

import concourse.bass as bass
import concourse.mybir as mybir
import concourse.tile as tile
from concourse import bacc
from concourse.bass_utils import run_bass_kernel_spmd

F32 = mybir.dt.float32

N = 16384           # nodes
NFEAT = 128         # input features
F1, F2, F3 = 32, 48, 64
NCORES = 8
R = N // NCORES     # rows (output nodes) per core = 2048
KT = N // 128       # k-tiles per layer = 128
MT = R // 128       # 128-row m-tiles per core = 16
MC = R // 512       # 512-col output chunks per core = 4
KC = 16             # DMA chunks per output chunk (8 k-tiles = 2 MiB each)
KB = KT // KC       # k-tiles per DMA chunk = 8

# Host-preswizzled adjacency: every (mc, kc) chunk is 2 MiB of fully
# sequential DRAM; within it, partition p holds, for kk in 0..7,
#   at_sw[mc, kc, p, kk, m] = adj[c*R + mc*512 + m, (kc*KB + kk)*128 + p]
AT_SHAPE = (MC * KC * 128, KB * 512)


def _ts(i, s):
    return slice(i * s, (i + 1) * s)


def _build_nc(reps=1):
    nc = bacc.Bacc(
        "TRN2", target_bir_lowering=False, debug=False, num_devices=NCORES
    )

    at = nc.dram_tensor("at", list(AT_SHAPE), F32, kind="ExternalInput")
    xt = nc.dram_tensor("xt", [NFEAT, N], F32, kind="ExternalInput")
    w1 = nc.dram_tensor("w1", [NFEAT, F1], F32, kind="ExternalInput")
    w2 = nc.dram_tensor("w2", [F1, F2], F32, kind="ExternalInput")
    w3 = nc.dram_tensor("w3", [F2, F3], F32, kind="ExternalInput")
    b1c = nc.dram_tensor("b1c", [F1, 1], F32, kind="ExternalInput")
    b2c = nc.dram_tensor("b2c", [F2, 1], F32, kind="ExternalInput")
    b3c = nc.dram_tensor("b3c", [F3, 1], F32, kind="ExternalInput")
    out = nc.dram_tensor("out", [F3, 1], F32, kind="ExternalOutput")

    # [MC, KC, 128, KB*512]
    at_r = at[:].rearrange("(mc kc p) q -> mc kc p q", kc=KC, p=128)

    with tile.TileContext(nc) as tc, ExitStack() as es:
        const = es.enter_context(tc.tile_pool(name="const", bufs=1))
        atp = es.enter_context(tc.tile_pool(name="atp", bufs=4))
        htp = es.enter_context(tc.tile_pool(name="htp", bufs=1))
        zlp = es.enter_context(tc.tile_pool(name="zlp", bufs=1))
        accp = es.enter_context(tc.tile_pool(name="accp", bufs=1))
        dram = es.enter_context(tc.tile_pool(name="dram", bufs=1, space="DRAM"))
        p_h = es.enter_context(tc.tile_pool(name="p_h", bufs=2, space="PSUM"))
        p_z = es.enter_context(tc.tile_pool(name="p_z", bufs=2, space="PSUM"))

        w1_sb = const.tile([NFEAT, F1], F32)
        w2_sb = const.tile([F1, F2], F32)
        w3_sb = const.tile([F2, F3], F32)
        b1_sb = const.tile([F1, 1], F32)
        b2_sb = const.tile([F2, 1], F32)
        b3_sb = const.tile([F3, 1], F32)

        nc.sync.dma_start(w1_sb[:], w1[:])
        nc.sync.dma_start(w2_sb[:], w2[:])
        nc.sync.dma_start(w3_sb[:], w3[:])
        nc.sync.dma_start(b1_sb[:], b1c[:])
        nc.sync.dma_start(b2_sb[:], b2c[:])
        nc.sync.dma_start(b3_sb[:], b3c[:])

        def z1_compute(z1_sb):
            XQ = 4096
            with tc.tile_pool(name="xtp", bufs=2) as xtp:
                for q in range(N // XQ):
                    xq_sb = xtp.tile([NFEAT, XQ], F32, tag="xq")
                    nc.sync.dma_start(xq_sb[:], xt[:, _ts(q, XQ)])
                    for jj in range(XQ // 128):
                        j = q * (XQ // 128) + jj
                        pz = p_z.tile([128, F1], F32, tag="pz")
                        nc.tensor.matmul(
                            pz[:], xq_sb[:, _ts(jj, 128)], w1_sb[:],
                            start=True, stop=True,
                        )
                        nc.vector.tensor_copy(z1_sb[:, j, :], pz[:])

        def layer(z_sb, f_in, b_sb, ht_sb, w_sb, f_out, znext_local, last):
            for mc in range(MC):
                ph = p_h.tile([f_in, 512], F32, tag="ph")
                for kc in range(KC):
                    a_sb = atp.tile([128, KB * 512], F32, tag="a")
                    nc.sync.dma_start(a_sb[:], at_r[mc, kc])
                    for kk in range(KB):
                        kt = kc * KB + kk
                        nc.tensor.matmul(
                            ph[:],
                            z_sb[:, kt, :],
                            a_sb[:, _ts(kk, 512)],
                            start=(kt == 0),
                            stop=(kt == KT - 1),
                        )
                # fused bias + relu, PSUM -> SBUF, bias along partitions
                nc.scalar.activation(
                    ht_sb[:, _ts(mc, 512)], ph[:],
                    mybir.ActivationFunctionType.Relu, bias=b_sb[:, 0:1],
                )
                if not last:
                    for ml in range(4):
                        mt = mc * 4 + ml
                        pz = p_z.tile([128, f_out], F32, tag="pz")
                        nc.tensor.matmul(
                            pz[:], ht_sb[:, _ts(mt, 128)], w_sb[:],
                            start=True, stop=True,
                        )
                        nc.vector.tensor_copy(znext_local[:, mt, :], pz[:])

        def gather(znl_sb, z_in, z_out, znext_sb, g):
            # z_in is [128, MT, g]: straight contiguous copy of the local z
            # tiles. AllGather concatenates rank blocks, so z_out is
            # [NCORES, 128, MT, g]; global k-tile (r*MT + mt) holds nodes
            # {r*R + mt*128 + p} — exactly the standard k-tiling.
            nc.sync.dma_start(z_in[:], znl_sb[:])
            nc.gpsimd.collective_compute(
                "AllGather",
                mybir.AluOpType.bypass,
                replica_groups=[list(range(NCORES))],
                ins=[z_in.opt()],
                outs=[z_out.opt()],
            )
            nc.sync.dma_start(
                znext_sb[:].rearrange("p (r mt) g -> p r mt g", r=NCORES),
                z_out[:].rearrange("r p mt g -> p r mt g"),
            )

        for _rep in range(reps):
            # collective bounce buffers (Shared tiles are single-write)
            z2_in = dram.tile([128, MT, F2], F32, tag=f"z2i{_rep}")
            z2_out = dram.tile([NCORES, 128, MT, F2], F32,
                               addr_space="Shared", tag=f"z2o{_rep}")
            z3_in = dram.tile([128, MT, F3], F32, tag=f"z3i{_rep}")
            z3_out = dram.tile([NCORES, 128, MT, F3], F32,
                               addr_space="Shared", tag=f"z3o{_rep}")

            # Pools released mid-trace must pop in LIFO order: open z3p
            # first (longest-lived), then z2p, then z1p.
            z3_es = ExitStack()
            z3p = z3_es.enter_context(tc.tile_pool(name="z3p", bufs=1))
            z2_es = ExitStack()
            z2p = z2_es.enter_context(tc.tile_pool(name="z2p", bufs=1))
            z1_es = ExitStack()
            z1p = z1_es.enter_context(tc.tile_pool(name="z1p", bufs=1))

            z1_sb = z1p.tile([128, KT, F1], F32)
            z1_compute(z1_sb)

            # ---- layer 1 ----
            h1t_sb = htp.tile([F1, R], F32, tag="h1t")
            z2l_sb = zlp.tile([128, MT, F2], F32, tag="z2l")
            z2_sb = z2p.tile([128, KT, F2], F32)
            layer(z1_sb, F1, b1_sb, h1t_sb, w2_sb, F2, z2l_sb, last=False)
            z1_es.close()
            gather(z2l_sb, z2_in, z2_out, z2_sb, F2)

            # ---- layer 2 ----
            h2t_sb = htp.tile([F2, R], F32, tag="h2t")
            z3l_sb = zlp.tile([128, MT, F3], F32, tag="z3l")
            z3_sb = z3p.tile([128, KT, F3], F32)
            layer(z2_sb, F2, b2_sb, h2t_sb, w3_sb, F3, z3l_sb, last=False)
            z2_es.close()
            gather(z3l_sb, z3_in, z3_out, z3_sb, F3)

            # ---- layer 3 + mean-pool partial ----
            h3t_sb = htp.tile([F3, R], F32, tag="h3t")
            layer(z3_sb, F3, b3_sb, h3t_sb, None, None, None, last=True)
            z3_es.close()

            red_sb = accp.tile([F3, 1], F32)
            nc.vector.tensor_reduce(
                red_sb[:], h3t_sb[:], mybir.AxisListType.X,
                mybir.AluOpType.add,
            )
        nc.sync.dma_start(out[:], red_sb[:])

    nc.compile()
    return nc


_NC_CACHE = {}


def _get_nc(reps=1):
    if reps not in _NC_CACHE:
        _NC_CACHE[reps] = _build_nc(reps)
    return _NC_CACHE[reps]


def make_in_maps(x, adj, W1, W2, W3, b1, b2, b3):
    x = np.ascontiguousarray(x, dtype=np.float32)
    xt = np.ascontiguousarray(x.T)
    common = {
        "xt": xt,
        "w1": np.ascontiguousarray(W1, dtype=np.float32),
        "w2": np.ascontiguousarray(W2, dtype=np.float32),
        "w3": np.ascontiguousarray(W3, dtype=np.float32),
        "b1c": np.ascontiguousarray(np.asarray(b1, np.float32).reshape(F1, 1)),
        "b2c": np.ascontiguousarray(np.asarray(b2, np.float32).reshape(F2, 1)),
        "b3c": np.ascontiguousarray(np.asarray(b3, np.float32).reshape(F3, 1)),
    }
    adj = np.asarray(adj, dtype=np.float32)
    in_maps = []
    for c in range(NCORES):
        blk = adj[c * R : (c + 1) * R, :]           # [R, N]
        # rows: mc*512 + m; cols: (kc*KB + kk)*128 + p
        # [mc, m, kc, kk, p] -> [mc, kc, p, kk, m]
        sw = blk.reshape(MC, 512, KC, KB, 128).transpose(0, 2, 4, 3, 1)
        at_c = np.ascontiguousarray(sw).reshape(AT_SHAPE)
        in_maps.append({"at": at_c, **common})
    return in_maps


def head(h3_sum, fcW1, fcb1, fcW2, fcb2):
    y = (h3_sum / np.float32(N)).astype(np.float32)
    y = np.maximum(y @ fcW1 + fcb1, np.float32(0.0))
    logits = y @ fcW2 + fcb2
    e = np.exp(logits - logits.max())
    return (e / e.sum()).astype(np.float32)


def kernel(
    x,
    adj,
    idx_map,  # unused by the reference model
    W1,
    b1,
    W2,
    b2,
    W3,
    b3,
    fcW1,
    fcb1,
    fcW2,
    fcb2,
):
    nc = _get_nc()
    in_maps = make_in_maps(x, adj, W1, W2, W3, b1, b2, b3)
    res = run_bass_kernel_spmd(nc, in_maps, core_ids=list(range(NCORES)))
    h3_sum = np.sum(
        [res.results[c]["out"][:, 0] for c in range(NCORES)], axis=0
    ).astype(np.float32)
    return head(h3_sum, fcW1, fcb1, fcW2, fcb2)


# revision 24
# speedup vs baseline: 1.9087x; 1.0947x over previous
"""Trainium2 Bass kernel for a 3-layer dense GCN (nn_GCN_13846974562486).

Math (reference):
    h1 = relu(adj @ (x  @ W1) + b1)   # [N, 32]
    h2 = relu(adj @ (h1 @ W2) + b2)   # [N, 48]
    h3 = relu(adj @ (h2 @ W3) + b3)   # [N, 64]
    y  = softmax(relu(mean(h3, 0) @ fcW1 + fcb1) @ fcW2 + fcb2)

Distribution: 1D row-shard of adj / output nodes over 8 cores. Each core
holds a host-preswizzled copy of adj[rows_c, :]^T (128 MiB) and computes
its 2048 output rows per layer transposed:
    hT[f, m] = relu( sum_k z[k, f] * adj[row_m, k] + b[f] )
as PE matmuls with the small z-tile [128, F] in the stationary slot and
the adjacency streamed 512-wide (measured ~2.3x faster than holding the
adjacency tile stationary, whose fused fp32 weight-load doesn't
pipeline). The host swizzle makes every 2 MiB DMA chunk a fully
sequential DRAM read (measured ~290-460 GB/s/core vs ~190-260 for the
naive strided pattern). adj is read once per layer — the memory-bound
term (3 x 128 MiB per core).

The transposed layout makes bias+relu a single fused ScalarEngine
activation from PSUM, z_next = h @ W_next a direct matmul (hT is already
the needed lhsT), and the mean-pool a free-dim reduce.

Between layers the [N, F] activations are AllGathered (1.5-2 MiB per
rank-block, contiguous layout). The 8 per-core partial sums [64] and the
tiny MLP head run on the host (exact fp32, ~100 kFLOP).
"""

import os
import sys

for _p in ("/opt/trn_rl_repo", "/root/.axon_site/_ro/trn_rl_repo"):
    if os.path.isdir(_p) and _p not in sys.path:
        sys.path.insert(0, _p)

from contextlib import ExitStack

import numpy as np

import concourse.bass as bass
import concourse.mybir as mybir
import concourse.tile as tile
from concourse import bacc
from concourse.bass_utils import run_bass_kernel_spmd

F32 = mybir.dt.float32

N = 16384           # nodes
NFEAT = 128         # input features
F1, F2, F3 = 32, 48, 64
NCORES = 8
R = N // NCORES     # rows (output nodes) per core = 2048
KT = N // 128       # k-tiles per layer = 128
MT = R // 128       # 128-row m-tiles per core = 16
MC = R // 512       # 512-col output chunks per core = 4
KC = 16             # DMA chunks per output chunk (8 k-tiles = 2 MiB each)
KB = KT // KC       # k-tiles per DMA chunk = 8

# Host-preswizzled adjacency: every (mc, kc) chunk is 2 MiB of fully
# sequential DRAM; within it, partition p holds, for kk in 0..7,
#   at_sw[mc, kc, p, kk, m] = adj[c*R + mc*512 + m, (kc*KB + kk)*128 + p]
AT_SHAPE = (MC * KC * 128, KB * 512)


def _ts(i, s):
    return slice(i * s, (i + 1) * s)


def _build_nc(reps=1):
    nc = bacc.Bacc(
        "TRN2", target_bir_lowering=False, debug=False, num_devices=NCORES
    )

    at = nc.dram_tensor("at", list(AT_SHAPE), F32, kind="ExternalInput")
    xt = nc.dram_tensor("xt", [NFEAT, N], F32, kind="ExternalInput")
    w1 = nc.dram_tensor("w1", [NFEAT, F1], F32, kind="ExternalInput")
    w2 = nc.dram_tensor("w2", [F1, F2], F32, kind="ExternalInput")
    w3 = nc.dram_tensor("w3", [F2, F3], F32, kind="ExternalInput")
    b1c = nc.dram_tensor("b1c", [F1, 1], F32, kind="ExternalInput")
    b2c = nc.dram_tensor("b2c", [F2, 1], F32, kind="ExternalInput")
    b3c = nc.dram_tensor("b3c", [F3, 1], F32, kind="ExternalInput")
    out = nc.dram_tensor("out", [F3, 1], F32, kind="ExternalOutput")

    # [MC, KC, 128, KB*512]
    at_r = at[:].rearrange("(mc kc p) q -> mc kc p q", kc=KC, p=128)

    with tile.TileContext(nc) as tc, ExitStack() as es:
        const = es.enter_context(tc.tile_pool(name="const", bufs=1))
        atp = es.enter_context(tc.tile_pool(name="atp", bufs=4))
        htp = es.enter_context(tc.tile_pool(name="htp", bufs=1))
        zlp = es.enter_context(tc.tile_pool(name="zlp", bufs=1))
        accp = es.enter_context(tc.tile_pool(name="accp", bufs=1))
        dram = es.enter_context(tc.tile_pool(name="dram", bufs=1, space="DRAM"))
        p_h = es.enter_context(tc.tile_pool(name="p_h", bufs=2, space="PSUM"))
        p_z = es.enter_context(tc.tile_pool(name="p_z", bufs=2, space="PSUM"))

        w1_sb = const.tile([NFEAT, F1], F32)
        w2_sb = const.tile([F1, F2], F32)
        w3_sb = const.tile([F2, F3], F32)
        b1_sb = const.tile([F1, 1], F32)
        b2_sb = const.tile([F2, 1], F32)
        b3_sb = const.tile([F3, 1], F32)

        nc.sync.dma_start(w1_sb[:], w1[:])
        nc.sync.dma_start(w2_sb[:], w2[:])
        nc.sync.dma_start(w3_sb[:], w3[:])
        nc.sync.dma_start(b1_sb[:], b1c[:])
        nc.sync.dma_start(b2_sb[:], b2c[:])
        nc.sync.dma_start(b3_sb[:], b3c[:])

        def z1_compute(z1_sb):
            XQ = 4096
            with tc.tile_pool(name="xtp", bufs=2) as xtp:
                for q in range(N // XQ):
                    xq_sb = xtp.tile([NFEAT, XQ], F32, tag="xq")
                    nc.sync.dma_start(xq_sb[:], xt[:, _ts(q, XQ)])
                    for jj in range(XQ // 128):
                        j = q * (XQ // 128) + jj
                        pz = p_z.tile([128, F1], F32, tag="pz")
                        nc.tensor.matmul(
                            pz[:], xq_sb[:, _ts(jj, 128)], w1_sb[:],
                            start=True, stop=True,
                        )
                        nc.vector.tensor_copy(z1_sb[:, j, :], pz[:])

        def layer(z_sb, f_in, b_sb, ht_sb, w_sb, f_out, znext_local, last):
            for mc in range(MC):
                ph = p_h.tile([f_in, 512], F32, tag="ph")
                for kc in range(KC):
                    a_sb = atp.tile([128, KB * 512], F32, tag="a")
                    nc.sync.dma_start(a_sb[:], at_r[mc, kc])
                    for kk in range(KB):
                        kt = kc * KB + kk
                        nc.tensor.matmul(
                            ph[:],
                            z_sb[:, kt, :],
                            a_sb[:, _ts(kk, 512)],
                            start=(kt == 0),
                            stop=(kt == KT - 1),
                        )
                # fused bias + relu, PSUM -> SBUF, bias along partitions
                nc.scalar.activation(
                    ht_sb[:, _ts(mc, 512)], ph[:],
                    mybir.ActivationFunctionType.Relu, bias=b_sb[:, 0:1],
                )
                if not last:
                    for ml in range(4):
                        mt = mc * 4 + ml
                        pz = p_z.tile([128, f_out], F32, tag="pz")
                        nc.tensor.matmul(
                            pz[:], ht_sb[:, _ts(mt, 128)], w_sb[:],
                            start=True, stop=True,
                        )
                        nc.vector.tensor_copy(znext_local[:, mt, :], pz[:])

        def gather(znl_sb, z_in, z_out, znext_sb, g):
            # z_in is [128, MT, g]: straight contiguous copy of the local z
            # tiles. AllGather concatenates rank blocks, so z_out is
            # [NCORES, 128, MT, g]; global k-tile (r*MT + mt) holds nodes
            # {r*R + mt*128 + p} — exactly the standard k-tiling.
            nc.sync.dma_start(z_in[:], znl_sb[:])
            nc.gpsimd.collective_compute(
                "AllGather",
                mybir.AluOpType.bypass,
                replica_groups=[list(range(NCORES))],
                ins=[z_in.opt()],
                outs=[z_out.opt()],
            )
            nc.sync.dma_start(
                znext_sb[:].rearrange("p (r mt) g -> p r mt g", r=NCORES),
                z_out[:].rearrange("r p mt g -> p r mt g"),
            )

        for _rep in range(reps):
            # collective bounce buffers (Shared tiles are single-write)
            z2_in = dram.tile([128, MT, F2], F32, tag=f"z2i{_rep}")
            z2_out = dram.tile([NCORES, 128, MT, F2], F32,
                               addr_space="Shared", tag=f"z2o{_rep}")
            z3_in = dram.tile([128, MT, F3], F32, tag=f"z3i{_rep}")
            z3_out = dram.tile([NCORES, 128, MT, F3], F32,
                               addr_space="Shared", tag=f"z3o{_rep}")

            # Pools released mid-trace must pop in LIFO order: open z3p
            # first (longest-lived), then z2p, then z1p.
            z3_es = ExitStack()
            z3p = z3_es.enter_context(tc.tile_pool(name="z3p", bufs=1))
            z2_es = ExitStack()
            z2p = z2_es.enter_context(tc.tile_pool(name="z2p", bufs=1))
            z1_es = ExitStack()
            z1p = z1_es.enter_context(tc.tile_pool(name="z1p", bufs=1))

            z1_sb = z1p.tile([128, KT, F1], F32)
            z1_compute(z1_sb)

            # ---- layer 1 ----
            h1t_sb = htp.tile([F1, R], F32, tag="h1t")
            z2l_sb = zlp.tile([128, MT, F2], F32, tag="z2l")
            z2_sb = z2p.tile([128, KT, F2], F32)
            layer(z1_sb, F1, b1_sb, h1t_sb, w2_sb, F2, z2l_sb, last=False)
            z1_es.close()
            gather(z2l_sb, z2_in, z2_out, z2_sb, F2)

            # ---- layer 2 ----
            h2t_sb = htp.tile([F2, R], F32, tag="h2t")
            z3l_sb = zlp.tile([128, MT, F3], F32, tag="z3l")
            z3_sb = z3p.tile([128, KT, F3], F32)
            layer(z2_sb, F2, b2_sb, h2t_sb, w3_sb, F3, z3l_sb, last=False)
            z2_es.close()
            gather(z3l_sb, z3_in, z3_out, z3_sb, F3)

            # ---- layer 3 + mean-pool partial ----
            h3t_sb = htp.tile([F3, R], F32, tag="h3t")
            layer(z3_sb, F3, b3_sb, h3t_sb, None, None, None, last=True)
            z3_es.close()

            red_sb = accp.tile([F3, 1], F32)
            nc.vector.tensor_reduce(
                red_sb[:], h3t_sb[:], mybir.AxisListType.X,
                mybir.AluOpType.add,
            )
        nc.sync.dma_start(out[:], red_sb[:])

    nc.compile()
    return nc


_NC_CACHE = {}


def _get_nc(reps=1):
    if reps not in _NC_CACHE:
        _NC_CACHE[reps] = _build_nc(reps)
    return _NC_CACHE[reps]


def make_in_maps(x, adj, W1, W2, W3, b1, b2, b3):
    x = np.ascontiguousarray(x, dtype=np.float32)
    xt = np.ascontiguousarray(x.T)
    common = {
        "xt": xt,
        "w1": np.ascontiguousarray(W1, dtype=np.float32),
        "w2": np.ascontiguousarray(W2, dtype=np.float32),
        "w3": np.ascontiguousarray(W3, dtype=np.float32),
        "b1c": np.ascontiguousarray(np.asarray(b1, np.float32).reshape(F1, 1)),
        "b2c": np.ascontiguousarray(np.asarray(b2, np.float32).reshape(F2, 1)),
        "b3c": np.ascontiguousarray(np.asarray(b3, np.float32).reshape(F3, 1)),
    }
    adj = np.asarray(adj, dtype=np.float32)
    in_maps = []
    for c in range(NCORES):
        blk = adj[c * R : (c + 1) * R, :]           # [R, N]
        # rows: mc*512 + m; cols: (kc*KB + kk)*128 + p
        # [mc, m, kc, kk, p] -> [mc, kc, p, kk, m]
        sw = blk.reshape(MC, 512, KC, KB, 128).transpose(0, 2, 4, 3, 1)
        at_c = np.ascontiguousarray(sw).reshape(AT_SHAPE)
        in_maps.append({"at": at_c, **common})
    return in_maps


def head(h3_sum, fcW1, fcb1, fcW2, fcb2):
    y = (h3_sum / np.float32(N)).astype(np.float32)
    y = np.maximum(y @ fcW1 + fcb1, np.float32(0.0))
    logits = y @ fcW2 + fcb2
    e = np.exp(logits - logits.max())
    return (e / e.sum()).astype(np.float32)


def kernel(
    x,
    adj,
    idx_map,  # unused by the reference model
    W1,
    b1,
    W2,
    b2,
    W3,
    b3,
    fcW1,
    fcb1,
    fcW2,
    fcb2,
):
    nc = _get_nc()
    in_maps = make_in_maps(x, adj, W1, W2, W3, b1, b2, b3)
    res = run_bass_kernel_spmd(nc, in_maps, core_ids=list(range(NCORES)))
    h3_sum = np.sum(
        [res.results[c]["out"][:, 0] for c in range(NCORES)], axis=0
    ).astype(np.float32)
    return head(h3_sum, fcW1, fcb1, fcW2, fcb2)
